# revision 5
# baseline (speedup 1.0000x reference)
"""BERT self-attention on 8 Trainium2 NeuronCores (Bass/Tile).

Problem: B=8, S=1024, H=1024, NH=16, HD=64, fp32.
Sharding: pure data-parallel — one batch element per core, weights
replicated. No collectives.

Math notes:
- The attention-mask bias broadcasts over keys ((1-mask)[...,None] is a
  per-(batch,query) constant added to every logit of a softmax row), so
  it cancels exactly in softmax for any finite mask. It is not used.
- Softmax is computed without max-subtraction: logits are ~N(0,1)
  (|max| < ~6), exp is comfortably within fp32 range.

Per-core algorithm (all matmuls in float32r — fp32 rounded to a 13-bit
mantissa by the producing op, full PE streaming rate at N>=256):
  XT[h,s]   = x^T               (PE transpose via identity, 64 blocks)
  WT[h,o]   = W^T               (per weight, 64 blocks)
  QT[o,s]   = Wq @ x^T + bq     (o on partitions -> per-partition bias)
  KT[o,s]   = Wk @ x^T + bk
  Vpad[s,o] = [x @ Wv^T + bv | 1]  (natural layout + ones column)
  per head pair (even head on partitions 0:64, odd on 64:128 — the two
  score matmuls target different PE row groups and run concurrently):
    scoresT[k,q] = KT_h^T-slice.T @ QT_h-slice   (K=d=64)
    E = exp(scoresT / 8)                          (ACT, psum->sbuf)
    pv[d+1,q]    = sum_k [V_h | 1]^T E            (K=k=128, M=65)
    ctxT -> PE-transpose -> [q, d+1]; ctx = pv[:,:64]*recip(pv[:,64]) + bv
"""
import numpy as np
from contextlib import ExitStack

import concourse.bass as bass
import concourse.tile as tile
from concourse import bacc, mybir
from concourse.bass_utils import run_bass_kernel_spmd
from concourse.masks import make_identity

B, S, H, NH = 8, 1024, 1024, 16
HD = H // NH          # 64
P = 128
NT = S // P           # 8 s-tiles
HT = H // P           # 8 h-tiles (contraction)
OT = H // P           # 8 o-tiles
QBS = 512             # q-block size
NQB = S // QBS        # 2 q-blocks
N_CORES = 8
F32 = mybir.dt.float32
F32R = mybir.dt.float32r
AF = mybir.ActivationFunctionType
ALU = mybir.AluOpType

_CACHE = {}


def _emit(tc):
    nc = tc.nc
    x = nc.dram_tensor("x", [S, H], F32, kind="ExternalInput").ap()
    wq = nc.dram_tensor("wq", [H, H], F32, kind="ExternalInput").ap()
    wk = nc.dram_tensor("wk", [H, H], F32, kind="ExternalInput").ap()
    wv = nc.dram_tensor("wv", [H, H], F32, kind="ExternalInput").ap()
    bq = nc.dram_tensor("bq", [H], F32, kind="ExternalInput").ap()
    bk = nc.dram_tensor("bk", [H], F32, kind="ExternalInput").ap()
    bv = nc.dram_tensor("bv", [H], F32, kind="ExternalInput").ap()
    out = nc.dram_tensor("out", [S, H], F32, kind="ExternalOutput").ap()

    with ExitStack() as top:
        consts = top.enter_context(tc.tile_pool(name="consts", bufs=1))
        big = top.enter_context(tc.tile_pool(name="big", bufs=1))

        ident = consts.tile([P, P], F32)
        make_identity(nc, ident[:])
        bq_sb = consts.tile([P, OT], F32, tag="bq")
        nc.sync.dma_start(bq_sb[:], bq.rearrange("(t p) -> p t", p=P))
        bk_sb = consts.tile([P, OT], F32, tag="bk")
        nc.sync.dma_start(bk_sb[:], bk.rearrange("(t p) -> p t", p=P))
        bv_row = consts.tile([1, H], F32, tag="bv_row")
        nc.sync.dma_start(bv_row[:], bv.unsqueeze(0))
        bv_bc = consts.tile([P, H], F32, tag="bv_bc")
        nc.gpsimd.partition_broadcast(bv_bc[:], bv_row[:])
        ones_f32 = consts.tile([P, P], F32, tag="ones")
        nc.vector.memset(ones_f32[:], 1.0)

        # long-lived activation tensors
        QT = big.tile([P, OT, S], F32R, tag="QT")    # QT[p, ot, s] = Q[s, ot*P+p]
        KT = big.tile([P, OT, S], F32R, tag="KT")
        Vpad = big.tile([P, NT, NH, HD + 1], F32R, tag="Vpad")

        # ---------------- phase 1+2: XT, then per-W transpose + projection
        with ExitStack() as qkv:
            nat = qkv.enter_context(tc.tile_pool(name="nat", bufs=4))
            xtp = qkv.enter_context(tc.tile_pool(name="xtp", bufs=1))
            wtp = qkv.enter_context(tc.tile_pool(name="wtp", bufs=1))
            tps = qkv.enter_context(tc.tile_pool(name="tps", bufs=4, space="PSUM"))
            mps = qkv.enter_context(tc.tile_pool(name="mps", bufs=2, space="PSUM"))

            XT = xtp.tile([P, HT, S], F32R)          # XT[p, ht, s] = x[s, ht*P+p]
            for st in range(NT):
                xn = nat.tile([P, H], F32, tag="nat")
                nc.sync.dma_start(xn[:], x.rearrange("(t p) h -> p t h", p=P)[:, st, :])
                for ht in range(HT):
                    pt = tps.tile([P, P], F32)
                    nc.tensor.transpose(pt[:], xn[:, ht * P:(ht + 1) * P], ident[:])
                    nc.vector.tensor_copy(XT[:, ht, st * P:(st + 1) * P], pt[:])

            def project(w_ap, kind):
                WT = wtp.tile([P, HT, H], F32R, tag="WT")  # WT[p, ht, o] = W[o, ht*P+p]
                for ot in range(OT):
                    wn = nat.tile([P, H], F32, tag="nat")
                    nc.sync.dma_start(
                        wn[:], w_ap.rearrange("(t p) h -> p t h", p=P)[:, ot, :])
                    for ht in range(HT):
                        pt = tps.tile([P, P], F32)
                        nc.tensor.transpose(pt[:], wn[:, ht * P:(ht + 1) * P], ident[:])
                        nc.vector.tensor_copy(WT[:, ht, ot * P:(ot + 1) * P], pt[:])
                if kind in ("q", "k"):
                    dst = QT if kind == "q" else KT
                    bias = bq_sb if kind == "q" else bk_sb
                    # QT[o-tile, s-block] = sum_ht WT-block.T @ XT-block
                    for ot in range(OT):
                        for sb in range(NQB):
                            pm = mps.tile([P, QBS], F32)
                            for ht in range(HT):
                                nc.tensor.matmul(
                                    pm[:],
                                    WT[:, ht, ot * P:(ot + 1) * P],
                                    XT[:, ht, sb * QBS:(sb + 1) * QBS],
                                    start=(ht == 0), stop=(ht == HT - 1))
                            nc.vector.tensor_scalar_add(
                                dst[:, ot, sb * QBS:(sb + 1) * QBS], pm[:],
                                bias[:, ot:ot + 1])
                else:
                    # V natural: V[s-tile, o-block] = sum_ht XT-block.T @ WT-block
                    for st in range(NT):
                        for ob in range(NQB):
                            pm = mps.tile([P, QBS], F32)
                            for ht in range(HT):
                                nc.tensor.matmul(
                                    pm[:],
                                    XT[:, ht, st * P:(st + 1) * P],
                                    WT[:, ht, ob * QBS:(ob + 1) * QBS],
                                    start=(ht == 0), stop=(ht == HT - 1))
                            nh0 = ob * (NH // NQB)  # 8 heads per o-block
                            nc.vector.tensor_tensor(
                                Vpad[:, st, nh0:nh0 + NH // NQB, 0:HD],
                                pm[:].rearrange("p (h d) -> p h d", d=HD),
                                bv_bc[:, ob * QBS:(ob + 1) * QBS].rearrange(
                                    "p (h d) -> p h d", d=HD),
                                ALU.add)
                    # ones column
                    nc.vector.tensor_copy(
                        Vpad[:, :, :, HD],
                        ones_f32[:].rearrange("p (a b) -> p a b", a=NT))

            project(wq, "q")
            project(wk, "k")
            project(wv, "v")

        # ---------------- phase 3: attention, head pairs
        with ExitStack() as att:
            ep = att.enter_context(tc.tile_pool(name="E", bufs=1))
            stp = att.enter_context(tc.tile_pool(name="stage", bufs=1))
            cp = att.enter_context(tc.tile_pool(name="ctxT", bufs=2))
            rp = att.enter_context(tc.tile_pool(name="recip", bufs=4))
            sps = att.enter_context(tc.tile_pool(name="sps", bufs=2, space="PSUM"))
            pvs = att.enter_context(tc.tile_pool(name="pvs", bufs=2, space="PSUM"))
            tp2 = att.enter_context(tc.tile_pool(name="tp2", bufs=2, space="PSUM"))

            stage = stp.tile([P, NT, NH, HD], F32, tag="stage")
            for ot in range(OT):          # head pair (2*ot, 2*ot+1)
                for qb in range(NQB):
                    E = ep.tile([P, NT, 2, QBS], F32R, tag="E")
                    for kt in range(NT):
                        ps_s = sps.tile([P, 2, QBS], F32, tag="s")
                        for j in range(2):        # j=0 even head, j=1 odd head
                            pr = slice(j * HD, (j + 1) * HD)
                            nc.tensor.matmul(
                                ps_s[:, j, :],
                                KT[pr, ot, kt * P:(kt + 1) * P],
                                QT[pr, ot, qb * QBS:(qb + 1) * QBS],
                                start=True, stop=True)
                        nc.scalar.activation(E[:, kt, :, :], ps_s[:],
                                             AF.Exp, scale=0.125)
                    for j in range(2):
                        h = 2 * ot + j
                        ps_pv = pvs.tile([HD + 1, QBS], F32, tag="pv")
                        for kt in range(NT):
                            nc.tensor.matmul(
                                ps_pv[:], Vpad[:, kt, h, :], E[:, kt, j, :],
                                start=(kt == 0), stop=(kt == NT - 1))
                        ctxT = cp.tile([HD + 1, QBS], F32, tag="ctxT")
                        nc.vector.tensor_copy(ctxT[:], ps_pv[:])
                        for c in range(QBS // P):   # 128-wide q chunks
                            pt = tp2.tile([P, HD + 1], F32, tag="t")
                            nc.tensor.transpose(
                                pt[:], ctxT[:, c * P:(c + 1) * P],
                                ident[:HD + 1, :HD + 1])
                            st = qb * (QBS // P) + c
                            rc = rp.tile([P, 1], F32, tag="rc")
                            nc.vector.reciprocal(rc[:], pt[:, HD:HD + 1])
                            nc.vector.scalar_tensor_tensor(
                                stage[:, st, h, :], pt[:, 0:HD], rc[:],
                                bv_bc[:, h * HD:(h + 1) * HD],
                                ALU.mult, ALU.add)

            # ---------------- output
            for st in range(NT):
                nc.sync.dma_start(
                    out.rearrange("(t p) o -> p t o", p=P)[:, st, :],
                    stage[:, st, :, :])


def build():
    if "nc" in _CACHE:
        return _CACHE["nc"]
    nc = bacc.Bacc("TRN2", target_bir_lowering=False, debug=False,
                   num_devices=N_CORES)
    with tile.TileContext(nc) as tc:
        _emit(tc)
    nc.compile()
    _CACHE["nc"] = nc
    return nc


def make_in_maps(hidden_state, Wq, bq, Wk, bk, Wv, bv):
    hs = np.ascontiguousarray(np.asarray(hidden_state, dtype=np.float32))
    common = {
        "wq": np.ascontiguousarray(np.asarray(Wq, np.float32)),
        "wk": np.ascontiguousarray(np.asarray(Wk, np.float32)),
        "wv": np.ascontiguousarray(np.asarray(Wv, np.float32)),
        "bq": np.ascontiguousarray(np.asarray(bq, np.float32)),
        "bk": np.ascontiguousarray(np.asarray(bk, np.float32)),
        "bv": np.ascontiguousarray(np.asarray(bv, np.float32)),
    }
    return [{"x": hs[i], **common} for i in range(N_CORES)]


def kernel(hidden_state, attention_mask, Wq, bq, Wk, bk, Wv, bv):
    # attention_mask: per-(batch, query) additive constant -> cancels in
    # softmax (see module docstring); unused.
    nc = build()
    in_maps = make_in_maps(hidden_state, Wq, bq, Wk, bk, Wv, bv)
    res = run_bass_kernel_spmd(nc, in_maps, list(range(N_CORES)))
    return np.stack([res.results[i]["out"] for i in range(N_CORES)], axis=0)
